# revision 36
# baseline (speedup 1.0000x reference)
"""CIF (continuous-integrate-and-fire) module kernel for Trainium2, SPMD over 8 cores.

Data-parallel over batch B=32 -> 4 rows/core. Per row:
  - fire_signal loaded as [t,d] tiles, PE-transposed to [d,t].
  - depthwise conv3 + residual via free-axis shifted fused multiply-adds.
  - LayerNorm folded into the dense GEMM (W2 = diag(ln_g)@wd_w - ones*c1/128,
    cb = ln_b@wd_w + wd_b), with x_post chunks as the stationary operand so
    outputs land [t-partition, j-free] and rstd is a per-partition scalar.
  - wp folded into the GEMM columns (sorted by sign of wp):
    alpha_pre = sum_{wp>0} max(r*G, m) + sum_{wp<0} min(r*G, m), m = -cb*wp,
    via two fused STT+accum ops per chunk (split across DVE and GPSIMD).
  - rsqrt via bit-trick seed + 3 Newton iterations; softplus/sigmoid built from
    Exp/Ln (single ACT table set); fire threshold compared pre-softplus.
  - cumsum via tensor_tensor_scan; searchsorted as ff[n] = sum_t (cum[t]<=n)
    using a one-hot row-broadcast matmul + is_le accumulate.
  - temporal/pitch/beat via indirect-DMA row gathers (empty slots hit a zero
    pad row); f1 replicated exactly via a host-built LUT; final projection via
    PE transposes + accumulated matmuls.

All small parameters/constants ship in one packed DRAM tensor (single DMA) to
keep per-instruction semaphore fan-in low.
"""

from contextlib import ExitStack

import numpy as np

import concourse.bass as bass
import concourse.bacc as bacc
import concourse.tile as tile  # noqa: F401
from concourse import mybir
from concourse.bass_utils import run_bass_kernel_spmd
from concourse.tile import TileContext

B, T, D = 32, 3000, 128
T_SW, D_SW = 375, 192
T_S1, D_S1 = 188, 192
N_MAX = 128
D_MODEL = 512
HID = 128
LN_EPS = 1e-5
THRESH = 1.0
TEMP = 0.1

NCORES = 8
R = B // NCORES
TP = 3072
NG = TP // 128
NC_ = R * NG
LUT_N = TP + 1

# mega-const column offsets
C_IDENT = 0
C_WCAT = 128
C_WCOLS = 257
C_IOTAN = 260
C_PREMASK = 261
C_MREP = 357
C_WPBC = 485
C_FTHR = 486
C_SIGB = 487
C_ZER = 488
C_ER = 489
C_ONESROW = 1001
C_PROJB = 1129
C_TLENF = 1641
C_PROJW = 1645
C_TOTAL = C_PROJW + 4 * D_MODEL  # 3693

F32 = mybir.dt.float32
I32 = mybir.dt.int32
Alu = mybir.AluOpType
Act = mybir.ActivationFunctionType

_CACHE = {}


def _build_program(ksplit: int):
    nc = bacc.Bacc()

    fire = nc.declare_dram_parameter("fire", [R, TP, D], F32, isOutput=False)
    ac = nc.declare_dram_parameter("ac", [R, T_SW, D_SW], F32, isOutput=False)
    s1 = nc.declare_dram_parameter("s1", [R, T_S1, D_S1], F32, isOutput=False)
    mega_d = nc.declare_dram_parameter("mega", [128, C_TOTAL], F32, isOutput=False)
    f1lut_d = nc.declare_dram_parameter("f1lut", [LUT_N, 1], I32, isOutput=False)

    embs_o = nc.declare_dram_parameter("embs", [R, N_MAX, D_MODEL], F32, isOutput=True)
    alpha_o = nc.declare_dram_parameter("alpha", [R, T], F32, isOutput=True)
    qty_o = nc.declare_dram_parameter("qty", [1, R], F32, isOutput=True)

    fire_flat = fire[:].rearrange("r t d -> (r t) d")
    ac_flat = ac[:].rearrange("r t d -> (r t) d")
    s1_flat = s1[:].rearrange("r t d -> (r t) d")

    with TileContext(nc) as tc, ExitStack() as ctx:
        consts = ctx.enter_context(tc.tile_pool(name="consts", bufs=1))
        bigA = ctx.enter_context(tc.tile_pool(name="bigA", bufs=2))
        bigB = ctx.enter_context(tc.tile_pool(name="bigB", bufs=2))
        small = ctx.enter_context(tc.tile_pool(name="small", bufs=2))
        jkp = ctx.enter_context(tc.tile_pool(name="jkp", bufs=4))
        tpsA = ctx.enter_context(tc.tile_pool(name="tpsA", bufs=2, space="PSUM"))
        gps = ctx.enter_context(tc.tile_pool(name="gps", bufs=2, space="PSUM"))
        psB = ctx.enter_context(tc.tile_pool(name="psB", bufs=2, space="PSUM"))
        ps96 = ctx.enter_context(tc.tile_pool(name="ps96", bufs=1, space="PSUM"))

        mega = consts.tile([128, C_TOTAL], F32)
        nc.sync.dma_start(out=mega[:], in_=mega_d[:])
        # wait-absorbers: give ACT and GPSIMD their mega-DMA wait on a tiny op
        # so later instructions stay within the per-instruction sync-wait limit
        ab = consts.tile([1, 4], F32)
        nc.scalar.activation(out=ab[0:1, 0:1], in_=mega[0:1, 0:1], func=Act.Copy, scale=1.0)
        nc.gpsimd.tensor_copy(out=ab[0:1, 1:2], in_=mega[0:1, 0:1])
        nc.vector.tensor_copy(out=ab[0:1, 2:3], in_=mega[0:1, 0:1])

        def mc(c0, w, p0=0, p=128):
            return mega[p0:p0 + p, c0:c0 + w]

        ident = mc(C_IDENT, 128)
        wcat = mc(C_WCAT, HID + 1)
        wcols = mc(C_WCOLS, 3)
        iotan = mc(C_IOTAN, 1)
        premask = mc(C_PREMASK, NC_)
        mrep = mc(C_MREP, HID)
        wpbc_col = mc(C_WPBC, 1)
        fthr_col = mc(C_FTHR, 1)
        sigb_col = mc(C_SIGB, 1)
        zcol4 = mc(C_ZER, 1, 0, R)
        er = mc(C_ER, R * 128, 0, R)
        onesrow = mc(C_ONESROW, 128, 0, 1)
        projb = mc(C_PROJB, D_MODEL, 0, 1)
        tlenf = mc(C_TLENF, R, 0, 1)
        projw = mc(C_PROJW, 4 * D_MODEL)

        sumpack = consts.tile([128, NC_], F32)
        sqpack = consts.tile([128, NC_], F32)
        apack = consts.tile([128, NC_], F32)
        qcols = consts.tile([128, R], F32)
        firerows = consts.tile([R, TP], F32)
        cumrows = consts.tile([R, TP], F32)
        alpharows = consts.tile([R, TP], F32)

        # ---------- per-row pipeline ----------
        for r in range(R):
            xnat = bigA.tile([128, NG, 128], F32, tag="xnat")
            nc.sync.dma_start(out=xnat[:], in_=fire[r].rearrange("(g p) j -> p g j", p=128))
            abx = tpsA.tile([1, 1], F32, tag="tps")
            nc.tensor.matmul(out=abx[:], lhsT=xnat[:, 0, 0:1], rhs=xnat[:, 0, 0:1],
                             start=True, stop=True)

            xT = bigA.tile([128, TP], F32, tag="xT")
            for grp in range(NG // 4):
                tp = tpsA.tile([128, 512], F32, tag="tps")
                for k in range(4):
                    g = grp * 4 + k
                    nc.tensor.transpose(out=tp[:, k * 128:(k + 1) * 128],
                                        in_=xnat[:, g, :], identity=ident)
                nc.scalar.activation(out=xT[:, grp * 512:(grp + 1) * 512], in_=tp[:], func=Act.Copy)

            t1 = bigA.tile([128, TP], F32, tag="xnat")
            nc.scalar.activation(out=t1[:], in_=xT[:], func=Act.Copy, scale=wcols[:, 1:2])
            nc.vector.scalar_tensor_tensor(
                out=t1[:, 1:TP], in0=xT[:, 0:TP - 1], scalar=wcols[:, 0:1],
                in1=t1[:, 1:TP], op0=Alu.mult, op1=Alu.add)
            xpost = bigB.tile([128, TP], F32, tag="xpost")
            nc.vector.memset(xpost[:, TP - 1:TP], 0.0)
            nc.vector.scalar_tensor_tensor(
                out=xpost[:, 0:TP - 1], in0=xT[:, 1:TP], scalar=wcols[:, 2:3],
                in1=t1[:, 0:TP - 1], op0=Alu.mult, op1=Alu.add)
            xsq = bigB.tile([128, TP], F32, tag="xsq")
            nc.scalar.activation(out=xsq[:], in_=xpost[:], func=Act.Square)
            abq = tpsA.tile([1, 1], F32, tag="tps")
            nc.tensor.matmul(out=abq[:], lhsT=xsq[:, 0:1], rhs=xsq[:, 0:1],
                             start=True, stop=True)

            gsb = bigB.tile([128, NG * (HID + 1)], F32, tag="gsb")
            for grp in range(NG // 3):
                gp = gps.tile([128, 3 * (HID + 1) + 3], F32, tag="gps")
                for k in range(3):
                    g = grp * 3 + k
                    nc.tensor.matmul(
                        out=gp[:, k * (HID + 1):(k + 1) * (HID + 1)],
                        lhsT=xpost[:, g * 128:(g + 1) * 128],
                        rhs=wcat, start=True, stop=True)
                    nc.tensor.matmul(
                        out=gp[:, 3 * (HID + 1) + k:3 * (HID + 1) + k + 1],
                        lhsT=xsq[:, g * 128:(g + 1) * 128],
                        rhs=wcat[:, HID:HID + 1], start=True, stop=True)
                nc.scalar.activation(
                    out=gsb[:, grp * 3 * (HID + 1):(grp + 1) * 3 * (HID + 1)],
                    in_=gp[:, 0:3 * (HID + 1)], func=Act.Copy)
                nc.scalar.activation(
                    out=sqpack[:, r * NG + grp * 3:r * NG + (grp + 1) * 3],
                    in_=gp[:, 3 * (HID + 1):3 * (HID + 1) + 3], func=Act.Copy)
            nc.vector.tensor_copy(
                out=sumpack[:, r * NG:(r + 1) * NG],
                in_=gsb[:].rearrange("p (g o) -> p g o", o=HID + 1)[:, :, HID:HID + 1])

            # stats -> rstd on [128, NG]
            cs = slice(r * NG, (r + 1) * NG)
            mu2 = small.tile([128, NG], F32, tag="st1")
            nc.scalar.activation(out=mu2[:], in_=sumpack[:, cs], func=Act.Square, scale=1.0 / D)
            vps = small.tile([128, NG], F32, tag="st2")
            nc.vector.scalar_tensor_tensor(
                out=vps[:], in0=sqpack[:, cs], scalar=1.0 / D, in1=mu2[:],
                op0=Alu.mult, op1=Alu.subtract)
            nc.vector.tensor_scalar(out=vps[:], in0=vps[:], scalar1=float(LN_EPS),
                                    scalar2=None, op0=Alu.add)
            seedi = small.tile([128, NG], I32, tag="st3")
            nc.vector.tensor_scalar(out=seedi[:], in0=vps[:].bitcast(I32), scalar1=1,
                                    scalar2=None, op0=Alu.logical_shift_right)
            nc.vector.tensor_scalar(out=seedi[:], in0=seedi[:], scalar1=-1,
                                    scalar2=0x5F3759DF, op0=Alu.mult, op1=Alu.add)
            rpk = small.tile([128, NG], F32, tag="st4")
            nc.vector.tensor_copy(out=rpk[:].bitcast(I32), in_=seedi[:])
            tmpa = small.tile([128, NG], F32, tag="st5")
            for _ in range(3):
                nc.vector.tensor_mul(out=tmpa[:], in0=rpk[:], in1=rpk[:])
                nc.vector.tensor_mul(out=tmpa[:], in0=tmpa[:], in1=vps[:])
                nc.vector.tensor_scalar(out=tmpa[:], in0=tmpa[:], scalar1=-0.5, scalar2=1.5,
                                        op0=Alu.mult, op1=Alu.add)
                nc.vector.tensor_mul(out=rpk[:], in0=rpk[:], in1=tmpa[:])

            # alpha pre-activation
            a1t = small.tile([128, NG], F32, tag="a1t")
            a2t = small.tile([128, NG], F32, tag="a2t")
            for g in range(NG):
                go = g * (HID + 1)
                eng = nc.vector
                jk = jkp.tile([128, HID], F32, tag="jkd")
                if ksplit > 0:
                    eng.scalar_tensor_tensor(
                        out=jk[:, 0:ksplit], in0=gsb[:, go:go + ksplit],
                        scalar=rpk[:, g:g + 1], in1=mrep[:, 0:ksplit],
                        op0=Alu.mult, op1=Alu.max, accum_out=a1t[:, g:g + 1])
                else:
                    nc.vector.memset(a1t[:, g:g + 1], 0.0)
                if ksplit < HID:
                    eng.scalar_tensor_tensor(
                        out=jk[:, ksplit:HID], in0=gsb[:, go + ksplit:go + HID],
                        scalar=rpk[:, g:g + 1], in1=mrep[:, ksplit:HID],
                        op0=Alu.mult, op1=Alu.min, accum_out=a2t[:, g:g + 1])
                else:
                    nc.vector.memset(a2t[:, g:g + 1], 0.0)
            nc.vector.tensor_add(out=apack[:, cs], in0=a1t[:], in1=a2t[:])

        # ---------- fire chain (packed) ----------
        firep = small.tile([128, NC_], F32, tag="fp1")
        nc.vector.scalar_tensor_tensor(
            out=firep[:], in0=apack[:], scalar=fthr_col, in1=premask,
            op0=Alu.is_gt, op1=Alu.mult)
        # softplus(x) = max(x,0) + log1p(exp(-|x|)), x = apack + wpbc
        ax = small.tile([128, NC_], F32, tag="sp1")
        nc.vector.tensor_scalar(out=ax[:], in0=apack[:], scalar1=wpbc_col,
                                scalar2=None, op0=Alu.add)
        p1 = small.tile([128, NC_], F32, tag="sp2")
        nc.vector.tensor_scalar(out=p1[:], in0=ax[:], scalar1=0.0, scalar2=None,
                                op0=Alu.max)
        nab = small.tile([128, NC_], F32, tag="sp3")
        nc.vector.scalar_tensor_tensor(out=nab[:], in0=p1[:], scalar=-2.0, in1=ax[:],
                                       op0=Alu.mult, op1=Alu.add)
        ex = small.tile([128, NC_], F32, tag="sp4")
        nc.scalar.activation(out=ex[:], in_=nab[:], func=Act.Exp)
        nc.vector.tensor_scalar(out=ex[:], in0=ex[:], scalar1=1.0, scalar2=None,
                                op0=Alu.add)
        aspk = small.tile([128, NC_], F32, tag="fp2")
        nc.scalar.activation(out=aspk[:], in_=ex[:], func=Act.Ln)
        nc.vector.tensor_add(out=aspk[:], in0=aspk[:], in1=p1[:])
        # sigmoid((aspk-1)/0.1) = 1/(1+exp(10-10*aspk))
        e2 = small.tile([128, NC_], F32, tag="sp5")
        nc.scalar.activation(out=e2[:], in_=aspk[:], func=Act.Exp,
                             scale=-1.0 / TEMP, bias=sigb_col)
        nc.vector.tensor_scalar(out=e2[:], in0=e2[:], scalar1=1.0, scalar2=None,
                                op0=Alu.add)
        onsp = small.tile([128, NC_], F32, tag="fp3")
        nc.vector.reciprocal(out=onsp[:], in_=e2[:])
        aspm = small.tile([128, NC_], F32, tag="fp4")
        nc.vector.tensor_mul(out=aspm[:], in0=aspk[:], in1=premask)
        onsm = small.tile([128, NC_], F32, tag="fp5")
        nc.vector.tensor_mul(out=onsm[:], in0=onsp[:], in1=premask)

        # qty
        for r in range(R):
            jk = jkp.tile([128, HID], F32, tag="jkd")
            nc.vector.tensor_scalar(
                out=jk[:, 0:NG], in0=onsm[:, r * NG:(r + 1) * NG], scalar1=1.0,
                scalar2=0.0, op0=Alu.mult, op1=Alu.add, accum_out=qcols[:, r:r + 1])
        qps = ps96.tile([1, R], F32, tag="qps")
        # partition reduction via the ones column of wcat
        nc.tensor.matmul(out=qps[:], lhsT=wcat[:, HID:HID + 1], rhs=qcols[:],
                         start=True, stop=True)
        qrow = small.tile([1, R], F32, tag="qrow")
        nc.vector.tensor_copy(out=qrow[:], in_=qps[:])
        qout = small.tile([1, R], F32, tag="qout")
        nc.vector.tensor_tensor(out=qout[:], in0=qrow[:], in1=tlenf, op=Alu.subtract)
        nc.scalar.activation(out=qout[:], in_=qout[:], func=Act.Abs)
        nc.sync.dma_start(out=qty_o[:], in_=qout[:])

        # transpose fire + masked alpha to row domain
        tp96 = ps96.tile([NC_, 256], F32, tag="tp96")
        nc.tensor.transpose(out=tp96[:, 0:128], in_=firep[:], identity=ident)
        nc.tensor.transpose(out=tp96[:, 128:256], in_=aspm[:], identity=ident)
        t96sb = small.tile([NC_, 256], F32, tag="t96")
        nc.scalar.activation(out=t96sb[:], in_=tp96[:], func=Act.Copy)
        nc.gpsimd.dma_start(
            out=firerows[:].rearrange("r (g p) -> r g p", p=128), in_=t96sb[:, 0:128])
        nc.gpsimd.dma_start(
            out=alpharows[:].rearrange("r (g p) -> r g p", p=128), in_=t96sb[:, 128:256])
        nc.sync.dma_start(out=alpha_o[:], in_=alpharows[:, 0:T])

        nc.vector.tensor_tensor_scan(
            out=cumrows[:], data0=firerows[:], data1=zcol4.to_broadcast([R, TP]),
            initial=0.0, op0=Alu.add, op1=Alu.add)

        # ---------- ff + gathers + projection ----------
        for r in range(R):
            ffparts = small.tile([128, 8], F32, tag="ffp")
            for n in range(6):
                bp = psB.tile([128, 512], F32, tag="bp")
                nc.tensor.matmul(
                    out=bp[:], lhsT=er[:, r * 128:(r + 1) * 128],
                    rhs=cumrows[:, n * 512:(n + 1) * 512], start=True, stop=True)
                w = 512 if n < 5 else T - 5 * 512
                jk = jkp.tile([128, 512], F32, tag="jkff")
                nc.vector.tensor_scalar(
                    out=jk[:, 0:w], in0=bp[:, 0:w], scalar1=iotan, scalar2=0.0,
                    op0=Alu.is_le, op1=Alu.add, accum_out=ffparts[:, n:n + 1])
            ffc = small.tile([128, 1], F32, tag="ffc")
            nc.vector.tensor_reduce(out=ffc[:], in_=ffparts[:, 0:6],
                                    axis=mybir.AxisListType.X, op=Alu.add)
            ffi = small.tile([128, 1], I32, tag="ffi")
            nc.vector.tensor_copy(out=ffi[:], in_=ffc[:])
            tidx = small.tile([128, 1], I32, tag="tidx")
            nc.vector.tensor_scalar(out=tidx[:], in0=ffi[:], scalar1=r * TP, scalar2=None,
                                    op0=Alu.add)
            f0i = small.tile([128, 1], I32, tag="f0i")
            nc.vector.tensor_scalar(out=f0i[:], in0=ffi[:], scalar1=T - 1, scalar2=None,
                                    op0=Alu.min)
            nc.vector.tensor_scalar(out=f0i[:], in0=f0i[:], scalar1=3, scalar2=None,
                                    op0=Alu.arith_shift_right)
            nc.vector.tensor_scalar(out=f0i[:], in0=f0i[:], scalar1=r * T_SW, scalar2=None,
                                    op0=Alu.add)
            f1i = small.tile([128, 1], I32, tag="f1i")
            nc.gpsimd.indirect_dma_start(
                out=f1i[:], out_offset=None, in_=f1lut_d[:],
                in_offset=bass.IndirectOffsetOnAxis(ap=ffi[:, 0:1], axis=0))
            nc.vector.tensor_scalar(out=f1i[:], in0=f1i[:], scalar1=r * T_S1, scalar2=None,
                                    op0=Alu.add)

            cat = small.tile([128, D_MODEL], F32, tag="cat")
            nc.gpsimd.indirect_dma_start(
                out=cat[:, 0:D], out_offset=None, in_=fire_flat,
                in_offset=bass.IndirectOffsetOnAxis(ap=tidx[:, 0:1], axis=0))
            nc.gpsimd.indirect_dma_start(
                out=cat[:, D:D + D_SW], out_offset=None, in_=ac_flat,
                in_offset=bass.IndirectOffsetOnAxis(ap=f0i[:, 0:1], axis=0))
            nc.gpsimd.indirect_dma_start(
                out=cat[:, D + D_SW:D_MODEL], out_offset=None, in_=s1_flat,
                in_offset=bass.IndirectOffsetOnAxis(ap=f1i[:, 0:1], axis=0))

            # absorb the three gather-DMA waits on PE with tiny matmuls
            for c0 in (0, D, D + D_SW):
                abp = tpsA.tile([1, 1], F32, tag="tps")
                nc.tensor.matmul(out=abp[:], lhsT=cat[:, c0:c0 + 1],
                                 rhs=cat[:, c0:c0 + 1], start=True, stop=True)
            ctp = tpsA.tile([128, 512], F32, tag="tps")
            for k in range(4):
                nc.tensor.transpose(out=ctp[:, k * 128:(k + 1) * 128],
                                    in_=cat[:, k * 128:(k + 1) * 128], identity=ident)
            ctsb = small.tile([128, 512], F32, tag="ctsb")
            nc.scalar.activation(out=ctsb[:], in_=ctp[:], func=Act.Copy)
            ep = psB.tile([128, D_MODEL], F32, tag="bp")
            for k in range(4):
                nc.tensor.matmul(out=ep[:], lhsT=ctsb[:, k * 128:(k + 1) * 128],
                                 rhs=projw[:, k * D_MODEL:(k + 1) * D_MODEL],
                                 start=(k == 0), stop=False)
            nc.tensor.matmul(out=ep[:], lhsT=onesrow, rhs=projb, start=False, stop=True)
            esb = small.tile([128, D_MODEL], F32, tag="esb")
            nc.scalar.activation(out=esb[:], in_=ep[:], func=Act.Copy)
            nc.sync.dma_start(out=embs_o[r], in_=esb[:])

    nc.compile()
    return nc


def _prep(conv_w, ln_g, ln_b, wd_w, wd_b, wp_w, wp_b, proj_w, proj_b):
    """Host-side parameter folding. Returns (ksplit, base mega array [128, C_TOTAL])."""
    f64 = np.float64
    W1 = ln_g.astype(f64)[:, None] * wd_w.astype(f64)
    c1 = W1.sum(0)
    W2 = W1 - c1[None, :] / D
    cb = ln_b.astype(f64) @ wd_w.astype(f64) + wd_b.astype(f64)
    wp = wp_w.astype(f64)[:, 0]
    perm = np.argsort(wp <= 0, kind="stable")
    wps = wp[perm]
    ksplit = int((wps > 0).sum())
    W2p = (W2[:, perm] * wps[None, :]).astype(np.float32)
    mrow = (-(cb[perm] * wps)).astype(np.float32)
    cbwp = float((cb * wp).sum())
    wpbc = np.float32(float(wp_b.astype(f64)[0]) + cbwp)
    cstar = np.float32(np.log(np.e - 1.0))
    fthr = np.float32(np.float64(cstar) - np.float64(wpbc))

    mega = np.zeros((128, C_TOTAL), np.float32)
    mega[:, C_IDENT:C_IDENT + 128] = np.eye(128, dtype=np.float32)
    mega[:, C_WCAT:C_WCAT + HID] = W2p
    mega[:, C_WCAT + HID] = 1.0
    mega[:, C_WCOLS + 0] = conv_w[:, 0, 0]
    mega[:, C_WCOLS + 1] = conv_w[:, 0, 1] + np.float32(1.0)
    mega[:, C_WCOLS + 2] = conv_w[:, 0, 2]
    mega[:, C_IOTAN] = np.arange(128, dtype=np.float32)
    mega[:, C_MREP:C_MREP + HID] = mrow[None, :]
    mega[:, C_WPBC] = wpbc
    mega[:, C_FTHR] = fthr
    mega[:, C_SIGB] = THRESH / TEMP
    for r in range(R):
        mega[r, C_ER + r * 128:C_ER + (r + 1) * 128] = 1.0
    mega[0, C_ONESROW:C_ONESROW + 128] = 1.0
    mega[0, C_PROJB:C_PROJB + D_MODEL] = proj_b.astype(np.float32)
    pw = np.ascontiguousarray(proj_w.astype(np.float32)).reshape(4, 128, D_MODEL)
    mega[:, C_PROJW:] = pw.transpose(1, 0, 2).reshape(128, 4 * D_MODEL)

    ffv = np.minimum(np.arange(LUT_N, dtype=np.float32), np.float32(T - 1))
    f1v = ((ffv * np.float32(T_S1)) / np.float32(T)).astype(np.int32)
    f1lut = np.clip(f1v, 0, T_S1 - 1).astype(np.int32).reshape(-1, 1)
    return ksplit, mega, f1lut


def kernel(fire_signal, acoustic_src, acoustic_src_s1, input_lengths, target_lengths,
           conv_w, ln_g, ln_b, wd_w, wd_b, wp_w, wp_b, proj_w, proj_b):
    fire_signal = np.asarray(fire_signal, np.float32)
    acoustic_src = np.asarray(acoustic_src, np.float32)
    acoustic_src_s1 = np.asarray(acoustic_src_s1, np.float32)
    input_lengths = np.asarray(input_lengths)
    target_lengths = np.asarray(target_lengths)

    ksplit, mega_base, f1lut = _prep(
        np.asarray(conv_w, np.float32), np.asarray(ln_g, np.float32),
        np.asarray(ln_b, np.float32), np.asarray(wd_w, np.float32),
        np.asarray(wd_b, np.float32), np.asarray(wp_w, np.float32),
        np.asarray(wp_b, np.float32), np.asarray(proj_w, np.float32),
        np.asarray(proj_b, np.float32))
    if ksplit not in _CACHE:
        _CACHE[ksplit] = _build_program(ksplit)
    nc = _CACHE[ksplit]

    tg = np.arange(128)[:, None] + 128 * (np.arange(NC_)[None, :] % NG)  # t per packed col
    in_maps = []
    for ci in range(NCORES):
        sl = slice(ci * R, (ci + 1) * R)
        fpad = np.zeros((R, TP, D), np.float32)
        fpad[:, :T, :] = fire_signal[sl]
        lens = input_lengths[sl].astype(np.int64)
        mega = mega_base.copy()
        lenspread = np.repeat(lens, NG)[None, :]  # [1, 96]
        mega[:, C_PREMASK:C_PREMASK + NC_] = (tg < lenspread).astype(np.float32)
        mega[0, C_TLENF:C_TLENF + R] = target_lengths[sl].astype(np.float32)
        in_maps.append({
            "fire": fpad,
            "ac": np.ascontiguousarray(acoustic_src[sl]),
            "s1": np.ascontiguousarray(acoustic_src_s1[sl]),
            "mega": mega,
            "f1lut": f1lut,
        })
    res = run_bass_kernel_spmd(nc, in_maps, core_ids=list(range(NCORES)))
    embs = np.concatenate([res.results[i]["embs"] for i in range(NCORES)], axis=0)
    alpha = np.concatenate([res.results[i]["alpha"] for i in range(NCORES)], axis=0)
    qty_parts = np.concatenate([res.results[i]["qty"][0] for i in range(NCORES)], axis=0)
    qty = np.float32(qty_parts.mean())
    return embs, alpha, qty
